# revision 10
# baseline (speedup 1.0000x reference)
"""Banded (stripe) attention weight kernel for Trainium2, SPMD over 8 NeuronCores.

Problem: scores = (Q @ K^T) / 16, masked to a per-sample diagonal stripe of
half-bandwidth 64 around the line col = row * (key_len/query_len), AND a key
validity mask; softmax over keys; returns (weights, value) with value passed
through unchanged.

Strategy: data-parallel over batch (2 samples/core). Only a narrow stripe of
each [1024, 4096] weight matrix is nonzero; per 128-row query tile the stripe
spans < 768 key columns (slope <= 4). The host pre-gathers transposed K
windows (snapped to a 128-column grid), precomputes the additive stripe/pad
mask and per-row scatter block indices; the device computes the banded
scores -> exp -> normalize and scatters only the nonzero 128-wide blocks into
the (pre-zeroed) output via indirect DMA. Everything else stays zero.
"""
import sys

for _p in ("/opt/trn_rl_repo",):
    if _p not in sys.path:
        sys.path.append(_p)

import numpy as np

B, LQ, LK, D = 16, 1024, 4096, 256
P = 128                 # partitions / query-tile rows
W = 768                 # key-window width (static)
NBLK = W // P           # 6 column blocks per window
QT = LQ // P            # 8 query tiles per sample
NCORES = 8
BPC = B // NCORES       # 2 samples per core
T = BPC * QT            # 16 tiles per core
HALF_BW = 64
SCALE = np.float32(0.0625)  # 1/16 = D**-0.5, exact power of two
NEG = -1e30
BLK_PER_ROW = LK // P   # 32
OUT_ROWS = BPC * LQ * BLK_PER_ROW  # 65536 blocks of 128 floats

_prog_cache = {}


# ---------------------------------------------------------------------------
# Workarounds: the staged walrus accepts only ONE sync-wait per instruction.
# ---------------------------------------------------------------------------

def _patch_tile_drain():
    import concourse.tile as tile
    from concourse.vector_clock import ScopedClock, VectorClock

    def _drain_and_barrier(self, tick_clock, wait_clock):
        nc = self.nc
        vclock = tick_clock.global_clock
        n = len(vclock)
        for proc in range(n):
            tick = vclock[proc]
            if tick <= 0:
                continue
            single = ScopedClock({None: VectorClock([0] * n)})
            single.require_at_least(None, proc, tick)
            nop = nc.sync.nop(nofuse=True)
            wait_clock.add_sem_waits(nop.ins, single)
            si = nop.ins.sync_info
            assert si is None or len(si.on_wait) <= 1
        nc.sync.drain()
        nc.all_engine_barrier()
        assert self.sems is not None
        popped = nc._tile_sem_poison_stack.pop()
        assert popped is self._sem_poison
        nc.clear_and_free_semaphores(list(self.sems.allocated().values()))
        nc.all_engine_barrier()

    tile.TileContext._drain_and_barrier = _drain_and_barrier


_split_n = [0]


def _split_multi_waits(nc):
    """Move all but the last sync-wait of any instruction onto single-wait
    NOPs inserted just before it on the same engine."""
    from concourse import mybir

    for fn in nc.m.functions:
        for bb in fn.blocks:
            insts = bb.instructions
            new = []
            changed = False
            for ins in insts:
                si = ins.sync_info
                if si is not None and len(si.on_wait) > 1:
                    waits = list(si.on_wait)
                    for w in waits[:-1]:
                        _split_n[0] += 1
                        nop = mybir.InstNoOp(
                            name=f"WSPLIT-{_split_n[0]}", ins=[], outs=[]
                        )
                        nop.engine = ins.engine
                        nop.sync_info = mybir.SyncInfo(on_wait=[w], on_update=[])
                        new.append(nop)
                    ins.sync_info = mybir.SyncInfo(
                        on_wait=[waits[-1]], on_update=list(si.on_update)
                    )
                    changed = True
                new.append(ins)
            if changed:
                bb.instructions = new


# ---------------------------------------------------------------------------
# Device program (SPMD; one NEFF shared by the 8 cores)
# ---------------------------------------------------------------------------

def _build_program():
    if "nc" in _prog_cache:
        return _prog_cache["nc"]
    from concourse import bass, mybir
    import concourse.tile as tile

    _patch_tile_drain()

    F32 = mybir.dt.float32
    BF16 = mybir.dt.bfloat16
    I32 = mybir.dt.int32
    AF = mybir.ActivationFunctionType

    nc = bass.Bass()
    # Per-core inputs (host pre-sharded / pre-gathered):
    #   qT:   [BPC, 2, P, LQ]  query^T (pre-scaled by 1/16), d split in 2 chunks
    #   kw:   [T, 2, P, W]     per-tile K window^T, d split in 2 chunks
    #   bias: [T, P, W]        additive mask (0 in stripe&valid, -1e30 outside)
    #   off:  [1, T]           per-tile window column offset k0 (int32)
    qT_d = nc.declare_dram_parameter("qT", [BPC, 2, P, LQ], F32, isOutput=False)
    kw_d = nc.declare_dram_parameter("kw", [T, 2, P, W], F32, isOutput=False)
    bias_d = nc.declare_dram_parameter("bias", [T, P, W], BF16, isOutput=False)
    off_d = nc.declare_dram_parameter("off", [1, T], I32, isOutput=False)
    out_d = nc.declare_dram_parameter("out", [BPC * LQ, LK], F32, isOutput=True)

    HW = W // 2  # 384-column halves -> one PSUM bank each

    with tile.TileContext(nc) as tc:
        with (
            tc.tile_pool(name="qt", bufs=1) as qt_pool,
            tc.tile_pool(name="kw", bufs=3) as kw_pool,
            tc.tile_pool(name="bias", bufs=3) as bias_pool,
            tc.tile_pool(name="work", bufs=3) as work_pool,
            tc.tile_pool(name="stat", bufs=4) as stat_pool,
            tc.tile_pool(name="psum", bufs=4, space="PSUM") as psum_pool,
        ):
            off_sb = qt_pool.tile([1, T], I32, tag="off")
            nc.sync.dma_start(out=off_sb[:], in_=off_d[:])
            # Load all query tiles once: 4 tiles of [P, LQ] (2 samples x 2 d-chunks)
            qts = {}
            for b in range(BPC):
                for c in range(2):
                    t_q = qt_pool.tile([P, LQ], F32, tag=f"qt{b}{c}")
                    nc.scalar.dma_start(out=t_q[:], in_=qT_d[b, c])
                    qts[(b, c)] = t_q

            for t in range(T):
                b, qt = divmod(t, QT)
                kw0 = kw_pool.tile([P, W], F32, tag="kw0")
                kw1 = kw_pool.tile([P, W], F32, tag="kw1")
                nc.sync.dma_start(out=kw0[:], in_=kw_d[t, 0])
                nc.scalar.dma_start(out=kw1[:], in_=kw_d[t, 1])
                bias = bias_pool.tile([P, W], BF16)
                nc.sync.dma_start(out=bias[:], in_=bias_d[t])

                s = work_pool.tile([P, W], F32, tag="s")
                kws = (kw0, kw1)
                qcol = slice(qt * P, (qt + 1) * P)
                for h in range(2):
                    ps = psum_pool.tile([P, HW], F32)
                    cols = slice(h * HW, (h + 1) * HW)
                    for c in range(2):
                        nc.tensor.matmul(
                            out=ps[:],
                            lhsT=qts[(b, c)][:, qcol],
                            rhs=kws[c][:, cols],
                            start=(c == 0),
                            stop=(c == 1),
                        )
                    # s = scores + bias (scale already folded into qT)
                    nc.vector.tensor_tensor(
                        out=s[:, cols], in0=ps[:], in1=bias[:, cols],
                        op=mybir.AluOpType.add,
                    )
                p = work_pool.tile([P, W], F32, tag="p")
                zsum = stat_pool.tile([P, 1], F32, tag="zsum")
                nc.scalar.activation(out=p[:], in_=s[:], func=AF.Exp,
                                     accum_out=zsum[:])
                zrec = stat_pool.tile([P, 1], F32, tag="zrec")
                nc.vector.reciprocal(out=zrec[:], in_=zsum[:])
                o = work_pool.tile([P, W], F32, tag="o")
                nc.scalar.activation(out=o[:], in_=p[:], func=AF.Copy,
                                     scale=zrec[:])
                # one dynamic-offset strided store: rows stride LK, cols at
                # k0. The host folds the row base into off so the offset is a
                # bare register (reg+const offsets fail to lower here).
                reg = nc.gpsimd.alloc_register(f"k0_{t}")
                nc.gpsimd.load(reg, off_sb[0:1, t:t + 1])
                val = nc.gpsimd.snap(
                    reg, min_val=0,
                    max_val=(BPC * LQ - P) * LK + LK - W,
                    guaranteed_mod_val=P)
                out_ap = bass.AP(out_d, val, [[LK, P], [1, W]])
                nc.gpsimd.dma_start(out=out_ap, in_=o[:])

    _split_multi_waits(nc)
    _prog_cache["nc"] = nc
    return nc


# ---------------------------------------------------------------------------
# Host-side preparation + execution
# ---------------------------------------------------------------------------

def _prepare_in_maps(query, key, mask, query_lengths, key_lengths):
    qf = np.ascontiguousarray(query, dtype=np.float32)
    kf = np.ascontiguousarray(key, dtype=np.float32)
    maskb = np.asarray(mask)
    ql = np.asarray(query_lengths).astype(np.float32)
    kl = np.asarray(key_lengths).astype(np.float32)

    # slope per sample, replicating the reference's f32 arithmetic exactly
    slope = (kl / ql).astype(np.float32)          # [B]

    # window start per (sample, query tile), snapped down to the 128 grid
    gx0 = (np.arange(QT, dtype=np.float32) * P)   # first row of each tile
    center0 = gx0[None, :] * slope[:, None]       # [B, QT] f32 exact products
    k0 = np.floor(center0).astype(np.int64) - HALF_BW
    k0 = (k0 // P) * P
    k0 = np.clip(k0, 0, LK - W).astype(np.int64)  # [B, QT]

    # coverage check: stripe end within window (stripe start >= k0 by constr.)
    gx_last = gx0 + (P - 1)
    center_last = (gx_last[None, :].astype(np.float32) * slope[:, None])
    hi_col = np.floor(center_last + HALF_BW).astype(np.int64)
    assert np.all(np.minimum(hi_col, LK - 1) <= k0 + W - 1), "window too narrow"
    lo_col = np.ceil(center0 - HALF_BW).astype(np.int64)
    assert np.all(np.maximum(lo_col, 0) >= k0), "window start too high"

    # per-(b,qt) bias tiles [B, QT, P, W], bit-exact with the reference mask
    gx = (np.arange(LQ, dtype=np.float32)
          .reshape(QT, P))                                    # [QT, P]
    center = gx[None] * slope[:, None, None]                  # [B, QT, P] f32
    gy = (k0[:, :, None, None].astype(np.float32)
          + np.arange(W, dtype=np.float32)[None, None, None, :])  # [B,QT,1,W]
    c4 = center[:, :, :, None]                                # [B, QT, P, 1]
    stripe = (gy >= c4 - np.float32(HALF_BW)) & (gy <= c4 + np.float32(HALF_BW))
    # key-validity mask gathered into the windows
    win_cols = k0[:, :, None] + np.arange(W)[None, None, :]   # [B, QT, W] int
    mask_w = np.take_along_axis(
        np.broadcast_to(maskb[:, None, :], (B, QT, LK)),
        win_cols, axis=2,
    )                                                         # [B, QT, W]
    valid = stripe & mask_w[:, :, None, :]
    import ml_dtypes
    bias = np.where(valid, np.float32(0.0), np.float32(NEG)).astype(
        ml_dtypes.bfloat16)                                   # [B, QT, P, W]

    # transposed, pre-scaled query: [B, 2, P, LQ]
    qT = (qf.transpose(0, 2, 1) * SCALE).reshape(B, 2, P, LQ)
    qT = np.ascontiguousarray(qT, dtype=np.float32)

    # K windows, transposed: [B, QT, 2, P, W]
    kT = np.ascontiguousarray(kf.transpose(0, 2, 1))          # [B, D, LK]
    kw = np.empty((B, QT, 2, P, W), np.float32)
    for bi in range(B):
        for qi in range(QT):
            s0 = int(k0[bi, qi])
            kwin = kT[bi, :, s0:s0 + W]                       # [D, W]
            kw[bi, qi, 0] = kwin[:P]
            kw[bi, qi, 1] = kwin[P:]

    # flat element offset of each tile's first row in the per-core output
    row_base = (np.arange(T) // QT * LQ + np.arange(T) % QT * P) * LK

    in_maps = []
    for core in range(NCORES):
        bs = slice(core * BPC, (core + 1) * BPC)
        in_maps.append({
            "qT": qT[bs],
            "kw": np.ascontiguousarray(kw[bs].reshape(T, 2, P, W)),
            "bias": np.ascontiguousarray(bias[bs].reshape(T, P, W)),
            "off": np.ascontiguousarray(
                (row_base + k0[bs].reshape(T)).reshape(1, T).astype(np.int32)),
        })
    return in_maps


def run(query, key, value, mask, query_lengths, key_lengths, trace=False):
    from concourse.bass_utils import run_bass_kernel_spmd

    nc = _build_program()
    in_maps = _prepare_in_maps(query, key, mask, query_lengths, key_lengths)
    res = run_bass_kernel_spmd(nc, in_maps, core_ids=list(range(NCORES)),
                               trace=trace)
    weight = np.empty((B, LQ, LK), np.float32)
    for core in range(NCORES):
        o = np.asarray(res.results[core]["out"]).reshape(BPC, LQ, LK)
        weight[core * BPC:(core + 1) * BPC] = o
    return weight, res


def kernel(query, key, value, mask, query_lengths, key_lengths):
    weight, _ = run(query, key, value, mask, query_lengths, key_lengths)
    return weight, np.asarray(value)


# revision 23
# speedup vs baseline: 1.1759x; 1.1759x over previous
"""Banded (stripe) attention weight kernel for Trainium2, SPMD over 8 NeuronCores.

Problem: scores = (Q @ K^T) / 16, masked to a per-sample diagonal stripe of
half-bandwidth 64 around the line col = row * (key_len/query_len), AND a key
validity mask; softmax over keys; returns (weights, value) with value passed
through unchanged.

Strategy: data-parallel over batch (2 samples/core). Only a narrow stripe of
each [1024, 4096] weight matrix is nonzero; per 128-row query tile the stripe
spans <= 641 key columns (slope <= 4), so a static 704-wide window (start
snapped to 64) covers it. The host pre-gathers transposed K windows,
precomputes the additive stripe/pad mask (bf16; its values 0 and -1e30 make
exp produce exact zeros), and per-tile flat output offsets; the device
computes the banded scores (scale folded into Q), adds the mask, does
exp-with-rowsum on the scalar engine, normalizes, and stores each tile's
[128, 704] window with one dynamic-offset strided DMA into the pre-zeroed
output. Everything outside the windows stays exactly zero.
"""
import sys

for _p in ("/opt/trn_rl_repo",):
    if _p not in sys.path:
        sys.path.append(_p)

import numpy as np

B, LQ, LK, D = 16, 1024, 4096, 256
P = 128                 # partitions / query-tile rows
W = 704                 # key-window width (static); band span <= 641+63
SNAP = 64               # window start alignment
QT = LQ // P            # 8 query tiles per sample
NCORES = 8
BPC = B // NCORES       # 2 samples per core
T = BPC * QT            # 16 tiles per core
HALF_BW = 64
SCALE = np.float32(0.0625)  # 1/16 = D**-0.5, exact power of two
NEG = -1e30

_prog_cache = {}

import os
KW_BUFS = int(os.environ.get("KW_BUFS", "2"))
BIAS_BUFS = int(os.environ.get("BIAS_BUFS", "2"))
WORK_BUFS = int(os.environ.get("WORK_BUFS", "2"))
PSUM_BUFS = int(os.environ.get("PSUM_BUFS", "2"))
O_BUFS = int(os.environ.get("O_BUFS", "2"))


# ---------------------------------------------------------------------------
# Workarounds: the staged walrus accepts only ONE sync-wait per instruction.
# ---------------------------------------------------------------------------

def _patch_tile_drain():
    import concourse.tile as tile
    from concourse.vector_clock import ScopedClock, VectorClock

    def _drain_and_barrier(self, tick_clock, wait_clock):
        nc = self.nc
        vclock = tick_clock.global_clock
        n = len(vclock)
        for proc in range(n):
            tick = vclock[proc]
            if tick <= 0:
                continue
            single = ScopedClock({None: VectorClock([0] * n)})
            single.require_at_least(None, proc, tick)
            nop = nc.sync.nop(nofuse=True)
            wait_clock.add_sem_waits(nop.ins, single)
            si = nop.ins.sync_info
            assert si is None or len(si.on_wait) <= 1
        nc.sync.drain()
        nc.all_engine_barrier()
        assert self.sems is not None
        popped = nc._tile_sem_poison_stack.pop()
        assert popped is self._sem_poison
        nc.clear_and_free_semaphores(list(self.sems.allocated().values()))
        nc.all_engine_barrier()

    tile.TileContext._drain_and_barrier = _drain_and_barrier


_split_n = [0]


def _split_multi_waits(nc):
    """Move all but the last sync-wait of any instruction onto single-wait
    NOPs inserted just before it on the same engine."""
    from concourse import mybir

    for fn in nc.m.functions:
        for bb in fn.blocks:
            insts = bb.instructions
            new = []
            changed = False
            for ins in insts:
                si = ins.sync_info
                if si is not None and len(si.on_wait) > 1:
                    waits = list(si.on_wait)
                    for w in waits[:-1]:
                        _split_n[0] += 1
                        nop = mybir.InstNoOp(
                            name=f"WSPLIT-{_split_n[0]}", ins=[], outs=[]
                        )
                        nop.engine = ins.engine
                        nop.sync_info = mybir.SyncInfo(on_wait=[w], on_update=[])
                        new.append(nop)
                    ins.sync_info = mybir.SyncInfo(
                        on_wait=[waits[-1]], on_update=list(si.on_update)
                    )
                    changed = True
                new.append(ins)
            if changed:
                bb.instructions = new


# ---------------------------------------------------------------------------
# Device program (SPMD; one NEFF shared by the 8 cores)
# ---------------------------------------------------------------------------

def _build_program(W=W):
    if W in _prog_cache:
        return _prog_cache[W]
    from concourse import bass, mybir
    import concourse.tile as tile

    _patch_tile_drain()

    F32 = mybir.dt.float32
    BF16 = mybir.dt.bfloat16
    I32 = mybir.dt.int32
    AF = mybir.ActivationFunctionType

    nc = bass.Bass()
    # Per-core inputs (host pre-sharded / pre-gathered):
    #   qT:   [BPC, 2, P, LQ]  query^T (pre-scaled by 1/16), d split in 2 chunks
    #   kw:   [T, P, 2W]       per-tile K window^T, d chunks side by side
    #   bias: [T, P, W]        additive mask (0 in stripe&valid, -1e30 outside)
    #   off:  [1, T]           per-tile flat output element offset (int32)
    qT_d = nc.declare_dram_parameter("qT", [BPC, 2, P, LQ], F32, isOutput=False)
    kw_d = nc.declare_dram_parameter("kw", [T, P, 2 * W], F32, isOutput=False)
    bias_d = nc.declare_dram_parameter("bias", [T, P, W], BF16, isOutput=False)
    off_d = nc.declare_dram_parameter("off", [1, T], I32, isOutput=False)
    out_d = nc.declare_dram_parameter("out", [BPC * LQ, LK], F32, isOutput=True)

    # split W into PSUM-bank-sized column chunks (<=512 f32 each)
    n_h = -(-W // 448)
    HW = (-(-W // (n_h * SNAP))) * SNAP
    h_cols = [(i * HW, min((i + 1) * HW, W)) for i in range(n_h)]

    with tile.TileContext(nc) as tc:
        with (
            tc.tile_pool(name="qt", bufs=1) as qt_pool,
            tc.tile_pool(name="kw", bufs=KW_BUFS) as kw_pool,
            tc.tile_pool(name="bias", bufs=BIAS_BUFS) as bias_pool,
            tc.tile_pool(name="work", bufs=WORK_BUFS) as work_pool,
            tc.tile_pool(name="outp", bufs=O_BUFS) as o_pool,
            tc.tile_pool(name="stat", bufs=6) as stat_pool,
            tc.tile_pool(name="psum", bufs=PSUM_BUFS, space="PSUM") as psum_pool,
        ):
            off_sb = qt_pool.tile([1, T], I32, tag="off")
            nc.sync.dma_start(out=off_sb[:], in_=off_d[:])
            # Query tiles load lazily: sample 0 up front, sample 1 mid-stream
            qts = {}

            def load_q(b):
                for c in range(2):
                    t_q = qt_pool.tile([P, LQ], F32, tag=f"qt{b}{c}")
                    nc.scalar.dma_start(out=t_q[:], in_=qT_d[b, c])
                    qts[(b, c)] = t_q

            load_q(0)
            for t in range(T):
                b, qt = divmod(t, QT)
                if t == QT // 2:
                    load_q(1)
                kw = kw_pool.tile([P, 2 * W], F32, tag="kw")
                if t % 2 == 0:
                    nc.sync.dma_start(out=kw[:], in_=kw_d[t])
                else:
                    nc.scalar.dma_start(out=kw[:], in_=kw_d[t])
                bias = bias_pool.tile([P, W], BF16)
                if t % 2 == 0:
                    nc.scalar.dma_start(out=bias[:], in_=bias_d[t])
                else:
                    nc.sync.dma_start(out=bias[:], in_=bias_d[t])

                s = work_pool.tile([P, W], F32, tag="s")
                qcol = slice(qt * P, (qt + 1) * P)
                for c0, c1 in h_cols:
                    ps = psum_pool.tile([P, c1 - c0], F32, tag="ps")
                    for c in range(2):
                        nc.tensor.matmul(
                            out=ps[:],
                            lhsT=qts[(b, c)][:, qcol],
                            rhs=kw[:, c * W + c0:c * W + c1],
                            start=(c == 0),
                            stop=(c == 1),
                        )
                    # s = scores + bias (scale already folded into qT)
                    nc.vector.tensor_tensor(
                        out=s[:, c0:c1], in0=ps[:], in1=bias[:, c0:c1],
                        op=mybir.AluOpType.add,
                    )
                p = work_pool.tile([P, W], F32, tag="p")
                zsum = stat_pool.tile([P, 1], F32, tag="zsum")
                nc.scalar.activation(out=p[:], in_=s[:], func=AF.Exp,
                                     accum_out=zsum[:])
                zrec = stat_pool.tile([P, 1], F32, tag="zrec")
                nc.vector.reciprocal(out=zrec[:], in_=zsum[:])
                o = o_pool.tile([P, W], F32, tag="o")
                nc.scalar.activation(out=o[:], in_=p[:], func=AF.Copy,
                                     scale=zrec[:])
                # one dynamic-offset strided store: rows stride LK, cols at
                # k0. The host folds the row base into off so the offset is a
                # bare register (reg+const offsets fail to lower here).
                reg = nc.gpsimd.alloc_register(f"k0_{t}")
                nc.gpsimd.load(reg, off_sb[0:1, t:t + 1])
                val = nc.gpsimd.snap(
                    reg, min_val=0,
                    max_val=(BPC * LQ - P) * LK + LK - W,
                    guaranteed_mod_val=SNAP)
                out_ap = bass.AP(out_d, val, [[LK, P], [1, W]])
                nc.gpsimd.dma_start(out=out_ap, in_=o[:])

    _split_multi_waits(nc)
    _prog_cache[W] = nc
    return nc


# ---------------------------------------------------------------------------
# Host-side preparation + execution
# ---------------------------------------------------------------------------

def _window_width(query_lengths, key_lengths):
    """Smallest SNAP-multiple window width covering every tile's stripe."""
    ql = np.asarray(query_lengths).astype(np.float32)
    kl = np.asarray(key_lengths).astype(np.float32)
    slope = (kl / ql).astype(np.float32)
    gx0 = np.arange(QT, dtype=np.float32) * P
    center0 = gx0[None, :] * slope[:, None]
    k0_raw = np.maximum(
        ((np.floor(center0).astype(np.int64) - HALF_BW) // SNAP) * SNAP, 0)
    center_last = (gx0 + (P - 1))[None, :].astype(np.float32) * slope[:, None]
    hi = np.minimum(np.floor(center_last).astype(np.int64) + HALF_BW, LK - 1)
    span = int((hi - k0_raw + 1).max())
    return max(-(-max(span, 1) // SNAP) * SNAP, 2 * SNAP)


def _prepare_in_maps(query, key, mask, query_lengths, key_lengths, W):
    qf = np.ascontiguousarray(query, dtype=np.float32)
    kf = np.ascontiguousarray(key, dtype=np.float32)
    maskb = np.asarray(mask)
    ql = np.asarray(query_lengths).astype(np.float32)
    kl = np.asarray(key_lengths).astype(np.float32)

    # slope per sample, replicating the reference's f32 arithmetic exactly
    slope = (kl / ql).astype(np.float32)          # [B]

    # window start per (sample, query tile), snapped down to the SNAP grid
    gx0 = (np.arange(QT, dtype=np.float32) * P)   # first row of each tile
    center0 = gx0[None, :] * slope[:, None]       # [B, QT] f32 exact products
    k0 = np.floor(center0).astype(np.int64) - HALF_BW
    k0 = (k0 // SNAP) * SNAP
    k0 = np.clip(k0, 0, LK - W).astype(np.int64)  # [B, QT]

    # coverage check: stripe end within window (stripe start >= k0 by constr.)
    gx_last = gx0 + (P - 1)
    center_last = (gx_last[None, :].astype(np.float32) * slope[:, None])
    hi_col = np.floor(center_last + HALF_BW).astype(np.int64)
    assert np.all(np.minimum(hi_col, LK - 1) <= k0 + W - 1), "window too narrow"
    lo_col = np.ceil(center0 - HALF_BW).astype(np.int64)
    assert np.all(np.maximum(lo_col, 0) >= k0), "window start too high"

    # per-(b,qt) bias tiles [B, QT, P, W], bit-exact with the reference mask
    gx = (np.arange(LQ, dtype=np.float32)
          .reshape(QT, P))                                    # [QT, P]
    center = gx[None] * slope[:, None, None]                  # [B, QT, P] f32
    gy = (k0[:, :, None, None].astype(np.float32)
          + np.arange(W, dtype=np.float32)[None, None, None, :])  # [B,QT,1,W]
    c4 = center[:, :, :, None]                                # [B, QT, P, 1]
    stripe = (gy >= c4 - np.float32(HALF_BW)) & (gy <= c4 + np.float32(HALF_BW))
    # key-validity mask gathered into the windows
    win_cols = k0[:, :, None] + np.arange(W)[None, None, :]   # [B, QT, W] int
    mask_w = np.take_along_axis(
        np.broadcast_to(maskb[:, None, :], (B, QT, LK)),
        win_cols, axis=2,
    )                                                         # [B, QT, W]
    valid = stripe & mask_w[:, :, None, :]
    import ml_dtypes
    bias = np.where(valid, np.float32(0.0), np.float32(NEG)).astype(
        ml_dtypes.bfloat16)                                   # [B, QT, P, W]

    # transposed, pre-scaled query: [B, 2, P, LQ]
    qT = (qf.transpose(0, 2, 1) * SCALE).reshape(B, 2, P, LQ)
    qT = np.ascontiguousarray(qT, dtype=np.float32)

    # K windows, transposed: [B, QT, 2, P, W]
    kT = np.ascontiguousarray(kf.transpose(0, 2, 1))          # [B, D, LK]
    kw = np.empty((B, QT, P, 2 * W), np.float32)
    for bi in range(B):
        for qi in range(QT):
            s0 = int(k0[bi, qi])
            kwin = kT[bi, :, s0:s0 + W]                       # [D, W]
            kw[bi, qi, :, :W] = kwin[:P]
            kw[bi, qi, :, W:] = kwin[P:]

    # flat element offset of each tile's first row in the per-core output
    row_base = (np.arange(T) // QT * LQ + np.arange(T) % QT * P) * LK

    in_maps = []
    for core in range(NCORES):
        bs = slice(core * BPC, (core + 1) * BPC)
        in_maps.append({
            "qT": qT[bs],
            "kw": np.ascontiguousarray(kw[bs].reshape(T, P, 2 * W)),
            "bias": np.ascontiguousarray(bias[bs].reshape(T, P, W)),
            "off": np.ascontiguousarray(
                (row_base + k0[bs].reshape(T)).reshape(1, T).astype(np.int32)),
        })
    return in_maps


def run(query, key, value, mask, query_lengths, key_lengths, trace=False):
    from concourse.bass_utils import run_bass_kernel_spmd

    Wn = max(_window_width(query_lengths, key_lengths), W)
    nc = _build_program(Wn)
    in_maps = _prepare_in_maps(query, key, mask, query_lengths, key_lengths, Wn)
    try:
        res = run_bass_kernel_spmd(nc, in_maps, core_ids=list(range(NCORES)),
                                   trace=trace)
    except (ModuleNotFoundError, ImportError):
        # A BASS_TRACE=1 environment routes through an NTFF profile hook
        # that this axon client build lacks; retry with tracing disabled.
        os.environ["BASS_NEVER_TRACE"] = "1"
        res = run_bass_kernel_spmd(nc, in_maps, core_ids=list(range(NCORES)),
                                   trace=False)
    weight = np.empty((B, LQ, LK), np.float32)
    for core in range(NCORES):
        o = np.asarray(res.results[core]["out"]).reshape(BPC, LQ, LK)
        weight[core * BPC:(core + 1) * BPC] = o
    return weight, res


def kernel(query, key, value, mask, query_lengths, key_lengths):
    weight, _ = run(query, key, value, mask, query_lengths, key_lengths)
    return weight, np.asarray(value)

